# revision 18
# baseline (speedup 1.0000x reference)
"""Farthest-point-sampling downsample kernel for Trainium2 (Bass/Tile).

Problem: xyz [8,3,32768], feature [8,128,32768] -> FPS-select 8192 points per
batch (exact fp32, first-occurrence argmax, seed point 0), return
(sampled_xyz [8,3,8192], sampled_feature [8,128,8192]).

Sharding: data-parallel over batch, one batch element per NeuronCore (8 cores).

Per-core layout: points p-major on SBUF as [128 partitions, 256 per partition]
(point i = p*256 + j). Each FPS step: ACT squares (scale=-1, bias=+centroid)
-> DVE adds/min -> DVE max/max_index (per-partition argmax) -> PE transpose of
packed (max, argmax, cand-x/y/z) -> DVE argmax over partitions -> mask-based
extraction of the winner's index and coords -> GPSIMD partition_broadcast of
the next centroid. Feature gather at the end via GPSIMD dma_gather on a
host-transposed [32768, 128] feature copy + PE transposes back.
"""

import numpy as np

import concourse.bass as bass
import concourse.mybir as mybir
from concourse.bass import ds
from concourse.bass_utils import run_bass_kernel_spmd
from concourse.masks import make_identity
from concourse.tile import TileContext
from concourse.vector_clock import ScopedClock
from concourse import library_config
from concourse.library_overlay import lower_extended_insts
import concourse.bass_isa as bass_isa

dt = mybir.dt
Alu = mybir.AluOpType

B = 8
P = 128
FREE = 256
N = P * FREE  # 32768
M = N // 4  # 8192
CIN = 128
UNROLL = 16
TOTAL_STEPS = M - 1  # argmax iterations (idx[0] is fixed to 0)

# ---------------------------------------------------------------------------
# Tail-drain patch: this walrus build's NO_STRUCT encoding fits only one sync
# wait per SP ctrl instruction; the Tile kernel-tail drain wants one wait per
# outstanding proc. Split them across SP nops (program order on SP preserves
# semantics).
_MAX_WAITS = 1


def _drain_and_barrier_split(self, tick_clock, wait_clock):
    nc = self.nc
    probe = nc.sync.nop(nofuse=True, hint="drain_wait_probe")
    wait_clock.add_sem_waits(probe.ins, ScopedClock({None: tick_clock.global_clock}))
    si = probe.ins.sync_info
    if si is not None and si.on_wait and len(si.on_wait) > _MAX_WAITS:
        waits = list(si.on_wait)
        si.on_wait = waits[:_MAX_WAITS]
        for k in range(_MAX_WAITS, len(waits), _MAX_WAITS):
            extra = nc.sync.nop(nofuse=True, hint=f"drain_wait_{k}")
            esi = extra.ins.sync_info
            if esi is None:
                esi = mybir.SyncInfo(on_update=[], on_wait=[])
                extra.ins.sync_info = esi
            esi.on_wait = waits[k : k + _MAX_WAITS]
    nc.sync.drain()
    nc.all_engine_barrier()
    assert self.sems is not None
    popped = nc._tile_sem_poison_stack.pop()
    assert popped is self._sem_poison
    nc.clear_and_free_semaphores(list(self.sems.allocated().values()))
    nc.all_engine_barrier()


TileContext._drain_and_barrier = _drain_and_barrier_split
# ---------------------------------------------------------------------------


def build_program(total_steps=TOTAL_STEPS, debug=False, skip_gather=False, timing_loops=1):
    nc = bass.Bass(num_swdge_queues=2)
    xyz_in = nc.dram_tensor("xyz", [3, P, FREE], dt.float32, kind="ExternalInput")
    featT_in = nc.dram_tensor("featT", [N, CIN], dt.float32, kind="ExternalInput")
    samp_xyz = nc.dram_tensor("samp_xyz", [3, M], dt.float32, kind="ExternalOutput")
    samp_feat = nc.dram_tensor(
        "samp_feat", [CIN, M], dt.float32, kind="ExternalOutput"
    )
    idx_out = nc.dram_tensor("idx16", [1, M], dt.int16, kind="ExternalOutput")
    if debug:
        dist_out = nc.dram_tensor("dist", [P, FREE], dt.float32, kind="ExternalOutput")

    loop_iters = total_steps // UNROLL
    epi = total_steps % UNROLL
    WQ = M // 16  # wrapped-layout block width (512)

    with TileContext(nc) as tc:
        with (
            tc.tile_pool(name="sb", bufs=1) as sb,
            tc.tile_pool(name="sfp", bufs=4) as sfp,
            tc.tile_pool(name="ps", bufs=1, space="PSUM") as ps,
            tc.tile_pool(name="pst", bufs=4, space="PSUM") as pst,
        ):
            # ------------------------- constants -------------------------
            jota256u = sb.tile([P, FREE], dt.uint32)
            nc.gpsimd.iota(jota256u, pattern=[[1, FREE]], base=0, channel_multiplier=0)
            jota256 = sb.tile([P, FREE], dt.float32)
            nc.vector.tensor_copy(jota256, jota256u)
            iota128u = sb.tile([1, P], dt.uint32)
            nc.gpsimd.iota(iota128u, pattern=[[1, P]], base=0, channel_multiplier=0)
            iota128 = sb.tile([1, P], dt.float32)
            nc.vector.tensor_copy(iota128, iota128u)
            ident = sb.tile([P, P], dt.float32)
            make_identity(nc, ident)
            ident1 = sb.tile([1, 1], dt.float32)
            nc.vector.memset(ident1, 1.0)

            nc.gpsimd.load_library(library_config.mlp)

            # ------------------------- state tiles -----------------------
            xyzpack = sb.tile([P, 3 * FREE], dt.float32)  # x | y | z
            sq = sb.tile([P, 3 * FREE], dt.float32)
            usum = sb.tile([P, FREE], dt.float32)
            dist = sb.tile([P, FREE], dt.float32)
            colmask = sb.tile([P, FREE], dt.float32)
            vj = sb.tile([P, 8], dt.float32)  # top8 maxes; cols 1:5 -> jf, cands
            jidx8 = sb.tile([P, 8], dt.uint32)
            vrowP = ps.tile([1, P], dt.float32)
            pmP = ps.tile([P, 1], dt.float32)
            pmS = sb.tile([P, 1], dt.float32)
            masked = sb.tile([P, 8], dt.float32)
            allred = sb.tile([P, 8], dt.float32)
            gmax8 = sb.tile([1, 8], dt.float32)
            gidx8 = sb.tile([1, 8], dt.uint32)
            pstarf = sb.tile([1, 1], dt.float32)
            jstarf = sb.tile([1, 1], dt.float32)
            posmask = sb.tile([1, P], dt.float32)
            scr128 = sb.tile([1, P], dt.float32)
            bias3 = sb.tile([P, 3], dt.float32)
            stage3 = sb.tile([1, 3 * UNROLL], dt.float32)  # coords per slot
            stage16 = sb.tile([1, UNROLL], dt.int16)  # idx per slot
            F = sb.tile([1, M], dt.int16)  # selected indices, WRAPPED layout:
            # F[(j%16)*WQ + j//16] = idx[j]  (dma_gather consumes this directly)
            F3 = sb.tile([1, 3 * M], dt.float32)  # sampled coords: c*M + j
            nc.vector.memset(F3[0:1, :], 0.0)
            Fw = sb.tile([P, M // 16], dt.int16)  # wrapped idx for dma_gather

            # ------------------------- prologue --------------------------
            for c in range(3):
                nc.gpsimd.dma_start(
                    out=xyzpack[:, c * FREE : (c + 1) * FREE], in_=xyz_in[c, :, :]
                )
            nc.vector.memset(dist, 1e10)
            nc.vector.memset(F[0:1, :], 0)  # pos 0 = idx[0] = 0; rest overwritten
            # idx[0] = 0 -> sampled coords = xyz[:, 0]
            for c in range(3):
                nc.vector.tensor_copy(
                    F3[0:1, c * M : c * M + 1], xyzpack[0:1, c * FREE : c * FREE + 1]
                )
            cent0 = F3[0:1, :].rearrange("p (c j) -> p c j", c=3)[0:1, :, 0:1]
            nc.gpsimd.partition_broadcast(allred[:, 2:5], cent0)

            # ------------------------- one FPS step ----------------------
            def emit_step(iv, u):
                # iv: For_i var or int (step 1); u: slot. j_out = iv*16 + u + 1
                xs = xyzpack[:, 0:FREE]
                ys = xyzpack[:, FREE : 2 * FREE]
                zs = xyzpack[:, 2 * FREE : 3 * FREE]
                sx = sq[:, 0:FREE]
                sy = sq[:, FREE : 2 * FREE]
                sz = sq[:, 2 * FREE : 3 * FREE]
                # (coord - centroid)^2 == (centroid - coord)^2, exact fp32
                bsrc = [allred[:, 2 + c : 3 + c] for c in range(3)]
                nc.scalar.activation(
                    sx, xs, mybir.ActivationFunctionType.Square,
                    bias=bsrc[0], scale=-1.0,
                )
                nc.scalar.activation(
                    sy, ys, mybir.ActivationFunctionType.Square,
                    bias=bsrc[1], scale=-1.0,
                )
                nc.scalar.activation(
                    sz, zs, mybir.ActivationFunctionType.Square,
                    bias=bsrc[2], scale=-1.0,
                )
                nc.vector.tensor_add(usum, sx, sy)
                nc.vector.tensor_add(usum, usum, sz)
                nc.vector.tensor_tensor(
                    out=dist, in0=dist, in1=usum, op=Alu.min
                )
                # per-partition argmax
                nc.vector.max(out=vj[:, 0:8], in_=dist)
                nc.vector.max_index(out=jidx8, in_max=vj[:, 0:8], in_values=dist)
                nc.vector.tensor_copy(vj[:, 1:2], jidx8[:, 0:1])  # u32 -> f32
                nc.vector.tensor_scalar(
                    colmask, jota256, vj[:, 1:2], None, op0=Alu.is_equal
                )
                # per-partition candidate coords of the argmax position
                for c, src in enumerate((xs, ys, zs)):
                    nc.vector.scalar_tensor_tensor(
                        out=sq[:, c * FREE : (c + 1) * FREE],
                        in0=src, scalar=0.0, in1=colmask,
                        op0=Alu.add, op1=Alu.mult,
                        accum_out=vj[:, 2 + c : 3 + c],
                    )
                # cross-partition argmax: transpose per-partition maxes to a row
                nc.tensor.transpose(vrowP[:, :], vj[:, 0:1], ident[:, :])
                nc.vector.max(out=gmax8, in_=vrowP[0:1, :])
                nc.vector.max_index(out=gidx8, in_max=gmax8, in_values=vrowP[0:1, :])
                nc.vector.tensor_copy(pstarf, gidx8[0:1, 0:1])
                nc.vector.tensor_scalar(
                    posmask, iota128, pstarf, None, op0=Alu.is_equal
                )
                # winner's packed row (jf, cands) -> all partitions, exactly:
                # mask partition p*, then partition-wise add-reduce broadcast
                nc.tensor.transpose(pmP[:, :], posmask[0:1, :], ident1[:, :])
                nc.vector.tensor_scalar(
                    masked, vj, pmP[:, 0:1], None, op0=Alu.mult
                )
                nc.gpsimd.partition_all_reduce(
                    allred[:, :], masked[:, :], P, bass_isa.ReduceOp.add
                )
                # coords (cols 2:5) -> slot-static staging; bias3 <- allred cols
                nc.vector.tensor_copy(
                    stage3[0:1, 3 * u : 3 * u + 3], allred[0:1, 2:5]
                )
                # idx = 256*p + j (i16 cast on write)
                nc.vector.scalar_tensor_tensor(
                    out=stage16[0:1, u : u + 1],
                    in0=pstarf, scalar=256.0, in1=allred[0:1, 1:2],
                    op0=Alu.mult, op1=Alu.add,
                )

            def flush_stages(iv):
                # move staged results for j = iv*16+1 .. iv*16+16 into F3/F
                # (few dynamic APs per body to stay within register budget)
                s3v = stage3[0:1, :].rearrange("p (u c) -> p c u", c=3)
                for c in range(3):
                    nc.vector.tensor_copy(
                        F3[0:1, ds(iv * 16 + (c * M + 1), UNROLL)],
                        s3v[0:1, c, :],
                    )
                # wrapped F positions: slots 0..14 -> (u+1)*WQ + iv; slot 15 -> iv+1
                fwv = F[0:1, :].rearrange("p (b q) -> p b q", q=WQ)
                nc.vector.tensor_copy(
                    fwv[0:1, 1:UNROLL, ds(iv, 1)],
                    stage16[0:1, 0 : UNROLL - 1].rearrange("p (u x) -> p u x", x=1),
                )
                nc.vector.tensor_copy(
                    F[0:1, ds(iv + 1, 1)], stage16[0:1, UNROLL - 1 : UNROLL]
                )

            # ------------------------- main loop -------------------------
            for _rep in range(timing_loops):
                if loop_iters > 0:
                    with tc.For_i(
                        0, loop_iters, 1, hint_engines=(mybir.EngineType.DVE,)
                    ) as i:
                        for u in range(UNROLL):
                            emit_step(i, u)
                        flush_stages(i)
            for u in range(epi):
                emit_step(loop_iters, u)
            if epi:
                # static flush of the partial epilogue body
                iv = loop_iters
                for u in range(epi):
                    j = iv * 16 + u + 1
                    for c in range(3):
                        nc.vector.tensor_copy(
                            F3[0:1, c * M + j : c * M + j + 1],
                            stage3[0:1, 3 * u + c : 3 * u + c + 1],
                        )
                    fp = (u + 1) * WQ + iv if u < UNROLL - 1 else iv + 1
                    nc.vector.tensor_copy(
                        F[0:1, fp : fp + 1], stage16[0:1, u : u + 1]
                    )

            # ------------------------- outputs ---------------------------
            nc.gpsimd.dma_start(out=samp_xyz[:, :], in_=F3[0:1, :])
            nc.gpsimd.dma_start(out=idx_out[0:1, :], in_=F[0:1, :])
            if debug:
                nc.gpsimd.dma_start(out=dist_out[:, :], in_=dist)

            # replicate wrapped idx to all 8 gpsimd-core partition groups
            for g in range(8):
                nc.gpsimd.dma_start(out=Fw[16 * g : 16 * (g + 1), :], in_=F[0:1, :])

            gath = sb.tile([P, (M // P) * CIN + 8], dt.float32)
            gview = gath[:, : (M // P) * CIN].rearrange("p (b c) -> p b c", c=CIN)
            if not skip_gather:
                # SWDGE descriptor buffer tops out ~1K entries per call:
                # gather in chunks of 1024 indices
                GCH = 1024
                nch = M // GCH
                gsem = nc.alloc_semaphore("gsem")
                with tc.tile_critical():
                    for k in range(nch):
                        nc.gpsimd.dma_gather(
                            out_ap=gview[:, (GCH // P) * k : (GCH // P) * (k + 1), :],
                            in_ap=featT_in[:, :],
                            idxs_ap=Fw[:, (GCH // 16) * k : (GCH // 16) * (k + 1)],
                            num_idxs=GCH,
                            num_idxs_reg=GCH,
                            elem_size=CIN,
                            queue_num=1,
                        ).then_inc(gsem, 16)
                    nc.gpsimd.wait_ge(gsem, 16 * nch)
                    # pad write: marks the tile written after DMA completion so
                    # downstream readers order after the actual data landing
                    nc.gpsimd.memset(gath[:, (M // P) * CIN :], 0)
            else:
                nc.vector.memset(gath[:, :], 0.0)
            for cblk in range(M // P):
                tp = pst.tile([P, P], dt.float32)
                nc.tensor.transpose(tp[:, :], gview[:, cblk, :], ident[:, :])
                tps = sfp.tile([P, P], dt.float32)
                nc.vector.tensor_copy(tps, tp[:, :])
                nc.gpsimd.dma_start(
                    out=samp_feat[:, cblk * P : (cblk + 1) * P], in_=tps
                )

    lower_extended_insts(nc)
    _split_sync_waits(nc)
    return nc


def _split_sync_waits(nc, max_waits=1):
    """This walrus build encodes at most one sync wait per instruction; move
    extra waits onto preceding same-engine nops (equivalent by program
    order)."""
    k = 0
    for f in nc.m.functions:
        for bb in f.blocks:
            new_insts = []
            for inst in bb.instructions:
                si = inst.sync_info
                if si is not None and si.on_wait and len(si.on_wait) > max_waits:
                    waits = list(si.on_wait)
                    for w in waits[max_waits:]:
                        nop = mybir.InstNoOp(
                            name=f"wsplit-{k}", ins=[], outs=[], engine=inst.engine
                        )
                        k += 1
                        nop.sync_info = mybir.SyncInfo(on_update=[], on_wait=[w])
                        nc.register_instruction(nop)
                        new_insts.append(nop)
                    si.on_wait = waits[:max_waits]
                new_insts.append(inst)
            if len(new_insts) != len(bb.instructions):
                bb.instructions[:] = new_insts


_PROGRAM = None


def _get_program():
    global _PROGRAM
    if _PROGRAM is None:
        _PROGRAM = build_program()
    return _PROGRAM


def kernel(xyz, feature):
    xyz = np.asarray(xyz, dtype=np.float32)
    feature = np.asarray(feature, dtype=np.float32)
    assert xyz.shape == (B, 3, N) and feature.shape == (B, CIN, N)
    nc = _get_program()
    in_maps = []
    for b in range(B):
        in_maps.append(
            {
                "xyz": np.ascontiguousarray(xyz[b]).reshape(3, P, FREE),
                "featT": np.ascontiguousarray(feature[b].T),
            }
        )
    res = run_bass_kernel_spmd(nc, in_maps, core_ids=list(range(B)))
    sampled = np.stack([r["samp_xyz"] for r in res.results])
    sampled_feature = np.stack([r["samp_feat"] for r in res.results])
    return sampled, sampled_feature
